# revision 25
# baseline (speedup 1.0000x reference)
"""Trainium2 Bass kernel for nn_CooccurrenceGraph (label co-occurrence graph attention).

Reference math (B=4096, N=80, H=256):
    q = x @ Wq.T + bq ; k = x @ Wk.T + bk ; v = x @ Wv.T + bv
    scores = (q @ k.T / 16) * cooc[None] * (labels*0.8+0.2)[:,None,:]
    attn = softmax(scores, -1)
    out = (attn @ v) @ Wo.T + bo

Strategy: pure data-parallel over 8 NeuronCores (512 batches each).
Per core, channel-major pipeline:
  - x shipped PRE-TRANSPOSED from host in two dtypes: fp8(e4m3) for the
    scores path and bf16 for the value path; label mask shipped
    pre-transposed [n, batch] as maskT. No device-side transposes.
  - Q' = Wq x^T + bq and K' = Wk x^T + bk channel-major via fp8 DoubleRow
    matmuls (contraction 256 = 2 k-tiles in one instruction); biases fused
    into the PSUM->SBUF evacuation; Q'/K' stored fp8.
  - scores_T[m,n] = K'_b.T Q'_b per batch in ONE fp8 DoubleRow matmul;
    multiplied by cooc^T/16 (DVE) and the per-partition label mask (GpSimd),
    Exp on ACT (scores are tiny, no max subtraction needed) -> e f16.
  - v/Wo folded on host: Wvo = Wo @ Wv; VO = x @ Wvo.T + bfin computed bf16
    in 128-token groups (full-partition evacuation), ones-augmented column
    so attn@VO also yields the softmax denominator. Batches whose 80 rows
    span two 128-row groups use two accumulating matmuls.
  - y = psum[:, :256] * recip(psum[:, 256]); output shipped bf16.
Engine balance: PSUM evacuation split DVE/ACT; SBUF-only mask-mul on GpSimd.
"""

import math
import os
import sys

sys.path.insert(0, "/opt/trn_rl_repo")

import ml_dtypes
import numpy as np

import concourse.bass as bass
import concourse.tile as tile
from concourse import bacc, mybir
from concourse.bass_utils import run_bass_kernel_spmd

B, N, H = 4096, 80, 256
N_CORES = 8
BS = B // N_CORES           # batches per core
GB = 32                     # batches per chunk
TOK = GB * N                # tokens per chunk (2560)
NG = TOK // 128             # 128-token groups per chunk (20)
SCALE = 1.0 / math.sqrt(H)
FQ = 512                    # q/k free-dim tile
NQ = TOK // FQ              # 5

F32 = mybir.dt.float32
F16 = mybir.dt.float16
BF16 = mybir.dt.bfloat16
F8 = mybir.dt.float8e4
NP_BF16 = ml_dtypes.bfloat16
NP_F8 = mybir.dt.np(F8)
DR = mybir.MatmulPerfMode.DoubleRow

_CACHE = {}


def _bcast(ap2, n, pos):
    """Insert a 0-stride dim of size n into a 2D AP at position pos (1 or 2)."""
    a = ap2.ap
    assert len(a) == 2
    if pos == 1:
        new = [a[0], [0, n], a[1]]
    else:
        new = [a[0], a[1], [0, n]]
    return bass.AP(tensor=ap2.tensor, offset=ap2.offset, ap=new)


def build(bs=BS, n_devices=N_CORES, reps=1, hwloop=False):
    """Build + compile the Bass program for `bs` batches per core.

    reps>1 re-runs the whole body (same I/O) for differential timing;
    hwloop=True wraps the reps in a hardware For_i loop (compact program)."""
    key = (bs, n_devices, reps, hwloop)
    if key in _CACHE:
        return _CACHE[key]

    assert bs % GB == 0
    nchunk = bs // GB
    ntok = bs * N

    nc = bacc.Bacc("TRN2", target_bir_lowering=False, debug=False,
                   enable_asserts=False, num_devices=n_devices)

    xt8_d = nc.dram_tensor("xt8", [H, ntok], F8, kind="ExternalInput").ap()
    xtb_d = nc.dram_tensor("xtb", [H, ntok], BF16, kind="ExternalInput").ap()
    maskT_d = nc.dram_tensor("maskT", [N, bs], F32, kind="ExternalInput").ap()
    wq_d = nc.dram_tensor("wq8", [H, H], F8, kind="ExternalInput").ap()
    wk_d = nc.dram_tensor("wk8", [H, H], F8, kind="ExternalInput").ap()
    wvo_d = nc.dram_tensor("wvoT", [H, H], BF16, kind="ExternalInput").ap()
    bq_d = nc.dram_tensor("bqr", [128, 2], F32, kind="ExternalInput").ap()
    bk_d = nc.dram_tensor("bkr", [128, 2], F32, kind="ExternalInput").ap()
    bfin_d = nc.dram_tensor("bfin", [128, H], F32, kind="ExternalInput").ap()
    cooc_d = nc.dram_tensor("coocT", [N, N], F32, kind="ExternalInput").ap()
    y_d = nc.dram_tensor("y", [ntok, H], BF16, kind="ExternalOutput").ap()

    with tile.TileContext(nc) as tc:
        with (
            tc.tile_pool(name="const", bufs=1) as constp,
            tc.tile_pool(name="xt", bufs=3) as xtp,
            tc.tile_pool(name="qk", bufs=2) as qkp,
            tc.tile_pool(name="vo", bufs=2) as vop,
            tc.tile_pool(name="yg", bufs=2) as ygp,
            tc.tile_pool(name="small", bufs=6) as smp,
            tc.tile_pool(name="psQK", bufs=3, space="PSUM") as psQK,
            tc.tile_pool(name="psS", bufs=2, space="PSUM") as psS,
            tc.tile_pool(name="psVY", bufs=3, space="PSUM") as psVY,
        ):
            # ---- constants (loaded once) ----
            wq_sb = constp.tile([128, 2, H], F8)     # [h_p, h_tile, o]
            wk_sb = constp.tile([128, 2, H], F8)
            wvo_sb = constp.tile([128, 2, H], BF16)
            nc.sync.dma_start(out=wq_sb, in_=wq_d.rearrange("(k p) o -> p k o", p=128))
            nc.sync.dma_start(out=wk_sb, in_=wk_d.rearrange("(k p) o -> p k o", p=128))
            nc.sync.dma_start(out=wvo_sb, in_=wvo_d.rearrange("(k p) o -> p k o", p=128))
            bq_sb = constp.tile([128, 2], F32)
            bk_sb = constp.tile([128, 2], F32)
            nc.sync.dma_start(out=bq_sb, in_=bq_d)
            nc.sync.dma_start(out=bk_sb, in_=bk_d)
            bfin_sb = constp.tile([128, H], F32)
            nc.sync.dma_start(out=bfin_sb, in_=bfin_d)
            bfin16_sb = constp.tile([128, H], BF16)
            nc.vector.tensor_copy(bfin16_sb, bfin_sb)
            cooc_sb = constp.tile([N, N], F32)
            nc.sync.dma_start(out=cooc_sb, in_=cooc_d)
            ones_sb = constp.tile([N, 1], BF16)
            nc.vector.memset(ones_sb, 1.0)

            def body(_iv=None):
              for c in range(nchunk):
                t0 = c * TOK
                # ---- x^T chunk, channel-major [h, tok] (pre-transposed)
                xt8 = xtp.tile([128, 2, TOK], F8, tag="xt8")
                xtb = xtp.tile([128, 2, TOK], BF16, tag="xtb")
                for k in range(2):
                    nc.sync.dma_start(
                        out=xt8[:, k, :],
                        in_=xt8_d[k * 128:(k + 1) * 128, t0:t0 + TOK])
                    nc.sync.dma_start(
                        out=xtb[:, k, :],
                        in_=xtb_d[k * 128:(k + 1) * 128, t0:t0 + TOK])
                # ---- label mask chunk [m, batch] (pre-transposed)
                maskT = smp.tile([N, GB], F32, tag="maskT")
                nc.sync.dma_start(out=maskT, in_=maskT_d[:, c * GB:(c + 1) * GB])

                # ---- Q' = Wq x^T + bq, K' = Wk x^T + bk (fp8 DoubleRow)
                q_sb = qkp.tile([128, 2, TOK], F8, tag="q")
                k_sb = qkp.tile([128, 2, TOK], F8, tag="k")
                for hf in range(NQ):
                    fsl = slice(hf * FQ, (hf + 1) * FQ)
                    for o in range(2):
                        osl = slice(o * 128, (o + 1) * 128)
                        psq = psQK.tile([128, FQ], F32, tag="ps_qk")
                        psk = psQK.tile([128, FQ], F32, tag="ps_qk")
                        nc.tensor.matmul(psq, wq_sb[:, :, osl],
                                         xt8[:, :, fsl], start=True, stop=True,
                                         perf_mode=DR)
                        nc.tensor.matmul(psk, wk_sb[:, :, osl],
                                         xt8[:, :, fsl], start=True, stop=True,
                                         perf_mode=DR)
                        if (hf * 2 + o) % 3 == 0:
                            nc.vector.tensor_scalar_add(
                                q_sb[:, o, fsl], psq, bq_sb[:, o:o + 1])
                        else:
                            nc.scalar.activation(
                                q_sb[:, o, fsl], psq,
                                mybir.ActivationFunctionType.Identity,
                                bias=bq_sb[:, o:o + 1])
                        if (hf * 2 + o) % 3 == 1:
                            nc.vector.tensor_scalar_add(
                                k_sb[:, o, fsl], psk, bk_sb[:, o:o + 1])
                        else:
                            nc.scalar.activation(
                                k_sb[:, o, fsl], psk,
                                mybir.ActivationFunctionType.Identity,
                                bias=bk_sb[:, o:o + 1])

                # ---- VO = x @ Wvo.T + bfin, token-major quads [m, 4, o] (bf16)
                vo_sb = vop.tile([N, GB, H], BF16, tag="vo")
                for bp in range(GB // 2):
                    psv = psVY.tile([N, 2, H], F32, tag="ps_vy")
                    for j in range(2):
                        b = bp * 2 + j
                        tsl = slice(b * N, (b + 1) * N)
                        nc.tensor.matmul(psv[:, j, :], xtb[:, 0, tsl],
                                         wvo_sb[:, 0, :], start=True, stop=False)
                        nc.tensor.matmul(psv[:, j, :], xtb[:, 1, tsl],
                                         wvo_sb[:, 1, :], start=False, stop=True)
                    vsl = slice(bp * 2, bp * 2 + 2)
                    if bp % 8 < 5:
                        nc.vector.tensor_add(vo_sb[:, vsl, :], psv,
                                             _bcast(bfin_sb[:N, :], 2, 1))
                    else:
                        nc.scalar.activation(
                            vo_sb[:, vsl, :], psv,
                            mybir.ActivationFunctionType.Identity)
                        nc.gpsimd.tensor_add(vo_sb[:, vsl, :], vo_sb[:, vsl, :],
                                             _bcast(bfin16_sb[:N, :], 2, 1))

                # ---- attention per group of 4 batches
                y_group = ygp.tile([N, GB, H], BF16, tag="yg")
                for g in range(GB // 4):
                    ps_s = psS.tile([N, 4, N], F32, tag="ps_s")
                    for j in range(4):
                        b = g * 4 + j
                        tsl = slice(b * N, (b + 1) * N)
                        nc.tensor.matmul(ps_s[:, j, :], k_sb[:, :, tsl],
                                         q_sb[:, :, tsl], start=True, stop=True,
                                         perf_mode=DR)
                    # scores_T * coocT/16, * mask[m] (per-partition, per-batch)
                    t2 = smp.tile([N, 4, N], F32, tag="t2")
                    nc.vector.tensor_mul(t2, ps_s, _bcast(cooc_sb, 4, 1))
                    nc.gpsimd.tensor_mul(
                        t2, t2, _bcast(maskT[:, g * 4:(g + 1) * 4], N, 2))
                    e4 = smp.tile([N, 4, N], BF16, tag="e4")
                    nc.scalar.activation(e4, t2, mybir.ActivationFunctionType.Exp)
                    # softmax denominators: rank-1 rsum into the dead ps_s col 0
                    for j in range(4):
                        nc.tensor.matmul(ps_s[:, j, 0:1], e4[:, j, :], ones_sb,
                                         start=True, stop=True,
                                         skip_group_check=True)
                    rc4 = smp.tile([N, 4], F32, tag="rc")
                    nc.vector.reciprocal(rc4, ps_s[:, :, 0])
                    for p in range(2):
                        ps_y = psVY.tile([N, 2, H], F32, tag="ps_vy")
                        for j in range(2):
                            b = g * 4 + p * 2 + j
                            nc.tensor.matmul(ps_y[:, j, :], e4[:, p * 2 + j, :],
                                             vo_sb[:, b, :], start=True, stop=True)
                        b0 = g * 4 + p * 2
                        if p == 0:
                            nc.vector.tensor_mul(
                                y_group[:, b0:b0 + 2, :], ps_y,
                                _bcast(rc4[:, p * 2:p * 2 + 2], H, 2))
                        else:
                            for j in range(2):
                                nc.scalar.activation(
                                    y_group[:, b0 + j, :], ps_y[:, j, :],
                                    mybir.ActivationFunctionType.Copy,
                                    scale=rc4[:, p * 2 + j:p * 2 + j + 1])

                # ---- store chunk output
                nc.sync.dma_start(
                    out=y_d[t0:t0 + TOK, :].rearrange("(b n) o -> n b o", n=N),
                    in_=y_group,
                )

            if hwloop and reps > 1:
                with tc.For_i(0, reps, 1) as _i:
                    body(_i)
            else:
                for rep in range(reps):
                    body()

    nc.compile()
    _CACHE[key] = nc
    return nc


def _prep_consts(Wq, bq, Wk, bk, Wv, bv, Wo, bo, cooccurrence):
    Wq = np.asarray(Wq, np.float32)
    Wk = np.asarray(Wk, np.float32)
    Wv = np.asarray(Wv, np.float32)
    Wo = np.asarray(Wo, np.float32)
    bv = np.asarray(bv, np.float32)
    bo = np.asarray(bo, np.float32)
    bq = np.asarray(bq, np.float32)
    bk = np.asarray(bk, np.float32)
    Wvo = Wo @ Wv                                  # vo = x @ Wvo.T
    bfin = Wo @ bv + bo
    # scores = q.k/16: fold nothing; fp8 weights, channel-major lhsT layout.
    return {
        "wq8": np.ascontiguousarray(Wq.T).astype(NP_F8),
        "wk8": np.ascontiguousarray(Wk.T).astype(NP_F8),
        "wvoT": np.ascontiguousarray(Wvo.T).astype(NP_BF16),
        "bqr": np.ascontiguousarray(bq.reshape(2, 128).T).astype(np.float32),
        "bkr": np.ascontiguousarray(bk.reshape(2, 128).T).astype(np.float32),
        "bfin": np.ascontiguousarray(np.broadcast_to(bfin, (128, H))).astype(np.float32),
        "coocT": np.ascontiguousarray(np.asarray(cooccurrence, np.float32).T * SCALE),
    }


def make_in_maps(inputs):
    """Per-core input dicts from the full reference inputs dict."""
    x = np.asarray(inputs["x"])
    labels = np.asarray(inputs["labels"])
    consts = _prep_consts(inputs["Wq"], inputs["bq"], inputs["Wk"], inputs["bk"],
                          inputs["Wv"], inputs["bv"], inputs["Wo"], inputs["bo"],
                          inputs["cooccurrence"])
    mask = (labels.astype(np.float32) * 0.8 + 0.2).reshape(B, N)
    x2 = x.reshape(B * N, H)
    in_maps = []
    for i in range(N_CORES):
        t0 = i * BS * N
        xT = np.ascontiguousarray(x2[t0:t0 + BS * N].T)
        in_maps.append({
            "xt8": xT.astype(NP_F8),
            "xtb": xT.astype(NP_BF16),
            "maskT": np.ascontiguousarray(mask[i * BS:(i + 1) * BS].T),
            **consts,
        })
    return in_maps


def kernel(x, Wq, bq, Wk, bk, Wv, bv, Wo, bo, cooccurrence, labels, _trace=False):
    nc = build()
    in_maps = make_in_maps(dict(x=x, Wq=Wq, bq=bq, Wk=Wk, bk=bk, Wv=Wv, bv=bv,
                                Wo=Wo, bo=bo, cooccurrence=cooccurrence,
                                labels=labels))
    try:
        res = run_bass_kernel_spmd(nc, in_maps, core_ids=list(range(N_CORES)),
                                   trace=_trace)
    except ModuleNotFoundError:
        res = run_bass_kernel_spmd(nc, in_maps, core_ids=list(range(N_CORES)),
                                   trace=False)
    out = np.concatenate([np.asarray(r["y"], np.float32) for r in res.results],
                         axis=0)
    ret = out.reshape(B, N, H)
    if _trace:
        kernel._last_results = res
    return ret
